# revision 57
# baseline (speedup 1.0000x reference)
"""FFF (fast feedforward / soft MoE tree) layer for Trainium2, 8 NeuronCores.

Strategy: data-parallel over the 4096-token batch (512 tokens/core), all
weights replicated. Per core, activations live feature-major in SBUF
([feature partitions, token free-dim]) so every matmul uses native weight
slices as lhsT and 512-token tiles as rhs:

  node phase:  hn^T = relu(W1n^T x^T + b1)          8 x 6 matmuls, N=512
               z    = W2bd^T hn^T                   8 matmuls (block-diag W2)
               c    = sigmoid(z + b2)
               w^T  = exp(Mpath^T ln([c; 1-c] + eps))  ACT chain + one fp32
                                                       path-matrix matmul
  leaf phase:  per leaf l: hl = relu(W1_l^T x^T + b1_l)   6 matmuls -> PSUM
               hls = hl * w_l (per-token scale via broadcast DMA of w rows)
               out^T += W2_l^T @ hls                 6 accumulating matmuls
               (+ leaf_b2 folded in as a rank-64 matmul over w^T)

out^T [768, 512] accumulates in 6 PSUM banks across all 64 leaves (4-leaf
software-pipeline skew keeps the PE saturated; the final leaves drain
bank-major so PSUM->SBUF copies overlap the last matmuls), then two batched
DMAs write DRAM; the host transposes and concatenates the 8 core shards.
Matmul inputs are bf16 (fp32 accumulation in PSUM); the path-matrix matmul
and all bias handling stay fp32.
"""

import functools
import os
import sys
from contextlib import ExitStack

import numpy as np
import ml_dtypes

for _p in ("/opt/trn_rl_repo", "/root/.axon_site/_ro/trn_rl_repo"):
    if os.path.isdir(_p) and _p not in sys.path:
        sys.path.insert(0, _p)

import concourse.bass as bass
import concourse.tile as tile
from concourse import bacc, mybir
from concourse.bass_utils import run_bass_kernel_spmd

BF16 = ml_dtypes.bfloat16

DEPTH = 6
IN_DIM = 768
NODE_HIDDEN = 16
LEAF_HIDDEN = 128
OUT_DIM = 768
BATCH = 4096
N_NODES = 63
N_LEAVES = 64
N_CORES = 8
BC = BATCH // N_CORES          # 512 tokens per core
KC = IN_DIM // 128             # 6 contraction chunks
HN = N_NODES * NODE_HIDDEN     # 1008 node-hidden total
NJ = (HN + 127) // 128         # 8 node-hidden partition tiles (last = 112)
OC = OUT_DIM // 128            # 6 output-feature chunks
GL = 4                         # leaves per weight-DMA group (fewer DMA issues)

# Exposed for test harnesses.
LAST_RESULT = None


def _path_matrix() -> np.ndarray:
    """Mpath [128, 64]: logw = Mpath^T @ [log(c) ; pad ; log(1-c) ; pad].

    c = sigmoid(z). Row n (0..62) selects log(c_n) for leaves in the LEFT
    subtree of node n; row 64+n selects log(1-c_n) for leaves in its RIGHT
    subtree (offset 64, not 63: engine APs must start on a partition
    quadrant). Rows 63 and 127 are zero.
    """
    m = np.zeros((128, N_LEAVES), np.float32)
    for leaf in range(N_LEAVES):
        for lvl in range(DEPTH):
            node = (1 << lvl) - 1 + (leaf >> (DEPTH - lvl))
            right = (leaf >> (DEPTH - 1 - lvl)) & 1
            m[node + (64 if right else 0), leaf] = 1.0
    return m


@functools.lru_cache(maxsize=1)
def _build_nc() -> bass.Bass:
    nc = bacc.Bacc()
    f32 = mybir.dt.float32
    bf16 = mybir.dt.bfloat16

    xt_d = nc.dram_tensor("xt", [128, KC, BC], bf16, kind="ExternalInput")
    w1n_d = nc.dram_tensor("w1n", [128, NJ, KC * 128], bf16, kind="ExternalInput")
    w2bd_d = nc.dram_tensor("w2bd", [128, NJ, N_NODES], bf16, kind="ExternalInput")
    b1n_d = nc.dram_tensor("b1n", [128, NJ], f32, kind="ExternalInput")
    b2sp_d = nc.dram_tensor("b2sp", [N_NODES, 2], f32, kind="ExternalInput")
    mneg_d = nc.dram_tensor("mneg", [128, N_LEAVES], f32, kind="ExternalInput")
    lw1_d = nc.dram_tensor(
        "lw1", [N_LEAVES // GL, 128, GL * KC * 128], bf16, kind="ExternalInput"
    )
    b1l_d = nc.dram_tensor("b1l", [128, N_LEAVES], f32, kind="ExternalInput")
    lw2_d = nc.dram_tensor(
        "lw2", [N_LEAVES // GL, 128, GL * OUT_DIM], bf16, kind="ExternalInput"
    )
    b2l_d = nc.dram_tensor("b2l", [N_LEAVES, OUT_DIM], bf16, kind="ExternalInput")
    out_d = nc.dram_tensor("outT", [OUT_DIM, BC], f32, kind="ExternalOutput")
    # Staging buffer so the per-token leaf weights can be broadcast-read
    # (partition-step-0 APs need a DRAM source).
    wt_dram = nc.dram_tensor("wt_scratch", [N_LEAVES, BC], bf16)

    act = mybir.ActivationFunctionType

    with tile.TileContext(nc) as tc, ExitStack() as ctx:
        consts = ctx.enter_context(tc.tile_pool(name="consts", bufs=1))
        wpool = ctx.enter_context(tc.tile_pool(name="wpool", bufs=3))
        apool = ctx.enter_context(tc.tile_pool(name="apool", bufs=2))
        ppool = ctx.enter_context(tc.tile_pool(name="ppool", bufs=2, space="PSUM"))
        opool = ctx.enter_context(tc.tile_pool(name="opool", bufs=1, space="PSUM"))

        # Every dma_start costs ~0.6us of serial issue time on the issuing
        # sequencer, so: x in ONE dma, node weights in 3 (j0 / j1 / j2-7 --
        # sized so each chunk lands just before PE needs it), everything not
        # needed immediately issued from the otherwise-idle GpSimd sequencer.
        xt = consts.tile([128, KC, BC], bf16)
        w1n = consts.tile([128, NJ, KC, 128], bf16)
        nc.sync.dma_start(out=xt[:, 0:1, :], in_=xt_d[:, 0:1, :])
        nc.sync.dma_start(
            out=w1n[:, 0, :, :],
            in_=w1n_d[:, 0, :].rearrange("p (c h) -> p c h", c=KC),
        )
        nc.sync.dma_start(out=xt[:, 1:3, :], in_=xt_d[:, 1:3, :])
        nc.sync.dma_start(out=xt[:, 3:, :], in_=xt_d[:, 3:, :])
        nc.sync.dma_start(
            out=w1n[:, 1, :, :],
            in_=w1n_d[:, 1, :].rearrange("p (c h) -> p c h", c=KC),
        )
        nc.sync.dma_start(
            out=w1n[:, 2:NJ, :, :],
            in_=w1n_d[:, 2:NJ, :].rearrange("p j (c h) -> p j c h", c=KC),
        )
        w2bd = consts.tile([128, NJ, N_NODES], bf16)
        nc.gpsimd.dma_start(out=w2bd, in_=w2bd_d[:])
        b1n = consts.tile([128, NJ], f32)
        nc.gpsimd.dma_start(out=b1n, in_=b1n_d[:])
        b2sp = consts.tile([N_NODES, 2], f32)
        nc.gpsimd.dma_start(out=b2sp, in_=b2sp_d[:])
        mneg = consts.tile([128, N_LEAVES], f32)
        nc.gpsimd.dma_start(out=mneg, in_=mneg_d[:])
        b1l = consts.tile([128, N_LEAVES], f32)
        nc.gpsimd.dma_start(out=b1l, in_=b1l_d[:])
        b2l = consts.tile([N_LEAVES, OUT_DIM], bf16)
        nc.gpsimd.dma_start(out=b2l, in_=b2l_d[:])

        hn = consts.tile([128, NJ, BC], bf16)
        pre = consts.tile([128, BC], f32)
        sp = consts.tile([128, BC], f32)
        wt = consts.tile([N_LEAVES, BC], bf16)
        # rows 63/127 of pre stay 1.0 -> ln gives 0 there, and Mpath's zero
        # rows ignore them
        nc.vector.memset(pre, 1.0)
        epsb = consts.tile([128, 1], f32)
        nc.vector.memset(epsb, 1e-38)

        # PE warmup: the HAM clock gate keeps an idle PE at 1.2 GHz and only
        # releases to 2.4 GHz after ~3.4us of sustained activity. The PE sits
        # idle waiting for the first DMAs anyway, so burn that window with
        # dummy 1x1 matmuls to arrive at the first real matmul already warm.
        warm = ppool.tile([128, BC], f32, tag="work", name="warm")
        for _ in range(8):
            nc.tensor.matmul(warm[:1, :], epsb, pre, start=True, stop=True)

        # ---- node phase: gate pre-activations z, then leaf weights w ----
        for j in range(NJ):
            pj = min(128, HN - j * 128)
            ph = ppool.tile([128, BC], f32, tag="work")
            for c in range(KC):
                nc.tensor.matmul(
                    ph[:pj, :],
                    w1n[:, j, c, :pj],
                    xt[:, c, :],
                    start=(c == 0),
                    stop=(c == KC - 1),
                )
            nc.scalar.activation(
                hn[:pj, j, :], ph[:pj, :], act.Relu, bias=b1n[:pj, j : j + 1]
            )

        zp = ppool.tile([128, BC], f32, tag="work")
        for j in range(NJ):
            pj = min(128, HN - j * 128)
            nc.tensor.matmul(
                zp[:N_NODES, :],
                w2bd[:pj, j, :],
                hn[:pj, j, :],
                start=(j == 0),
                stop=(j == NJ - 1),
            )
        # ---- leaf-phase pipeline helpers ----
        pouts = [
            opool.tile([128, BC], f32, tag=f"out{o}", name=f"pout{o}")
            for o in range(OC)
        ]
        wreps = {}

        def emit_wrep_dma(grp):
            """Broadcast leaf-weight rows (4 leaves) across all partitions."""
            wrep = wpool.tile([128, GL, BC], bf16, tag="wrep", bufs=3, name="wrep")
            src = bass.AP(
                tensor=wt_dram,
                offset=grp * GL * BC,
                ap=[[0, 128], [BC, GL], [1, BC]],
            )
            nc.sync.dma_start(out=wrep, in_=src)
            wreps[grp] = wrep

        lwg = {}

        def emit_lw1_dma(g):
            w1t = wpool.tile([128, GL, KC, 128], bf16, tag="lw1", bufs=2, name="w1t")
            nc.sync.dma_start(
                out=w1t,
                in_=lw1_d[g].rearrange("p (i c h) -> p i c h", i=GL, c=KC),
            )
            return w1t

        def emit_lw2_dma(g):
            w2t = wpool.tile([128, GL, OUT_DIM], bf16, tag="lw2", bufs=2, name="w2t")
            nc.sync.dma_start(
                out=w2t, in_=lw2_d[g].rearrange("p (i o) -> p i o", i=GL)
            )
            return w2t

        def front_a(leaf, defer_lw2=False):
            """Weight DMAs (grouped) + hl matmuls + relu for one leaf."""
            if leaf % GL == 2 and leaf >= GL and leaf + 2 < N_LEAVES:
                # prefetch the NEXT group's broadcast two leaves early
                # (groups 0/1 are emitted explicitly after wt_dram is
                # written; emission order carries the RAW dep on wt_dram)
                emit_wrep_dma(leaf // GL + 1)
            g = leaf // GL
            if leaf % GL == 0:
                lwg[g] = [emit_lw1_dma(g), None if defer_lw2 else emit_lw2_dma(g)]
            w1t = lwg[g][0]
            i = leaf % GL

            ph = ppool.tile([128, BC], f32, tag="work", name="ph")
            for c in range(KC):
                nc.tensor.matmul(
                    ph,
                    w1t[:, i, c, :],
                    xt[:, c, :],
                    start=(c == 0),
                    stop=(c == KC - 1),
                )
            hl = apool.tile([128, BC], bf16, tag="hl", bufs=6, name="hl")
            nc.scalar.activation(hl, ph, act.Relu, bias=b1l[:, leaf : leaf + 1])
            return (hl, leaf)

        def front_b(st):
            """Per-token leaf-weight scale (needs wrep for the leaf's group)."""
            hl, leaf = st
            hls = apool.tile([128, BC], bf16, tag="hls", bufs=7, name="hls")
            nc.vector.tensor_mul(hls, hl, wreps[leaf // GL][:, leaf % GL, :])
            return (hls, leaf)

        def leaf_out(pend, last=False):
            p_hls, p_leaf = pend
            p_w2t = lwg[p_leaf // GL][1]
            for o in range(OC):
                nc.tensor.matmul(
                    pouts[o],
                    p_w2t[:, p_leaf % GL, o * 128 : (o + 1) * 128],
                    p_hls,
                    start=False,
                    stop=last,
                )

        # The gating ACT/DVE chain is emitted BEFORE the prefilled leaves so
        # it isn't queued behind their relu ops on the Scalar engine.
        # pre[0:63]  = c = sigmoid(zp + b2);  pre[64:127] = 1 - c
        # sp = ln(pre + 1e-38) in ONE activation over all 128 partitions --
        # the +eps bias keeps saturated gates finite (ln(1e-38) = -87.5,
        # whose exp underflows to the correct 0 leaf weight, and never
        # produces inf/NaN in the path matmul).
        nc.scalar.activation(
            pre[0:N_NODES, :], zp[:N_NODES, :], act.Sigmoid, bias=b2sp[:, 1:2]
        )
        nc.vector.tensor_scalar(
            pre[64 : 64 + N_NODES, :], pre[0:N_NODES, :], -1.0, 1.0,
            mybir.AluOpType.mult, mybir.AluOpType.add,
        )
        nc.scalar.activation(sp, pre, act.Ln, bias=epsb)

        # Prefill leaves: their hl matmuls keep PE busy while the gating
        # chain (sigmoid/ln table loads -> path matmul -> exp -> DRAM round
        # trip for the broadcast) produces the leaf weights. Group 0's
        # second-matmul weights are deferred so the wt round trip doesn't
        # queue behind their transfer.
        prefill = [front_a(0, defer_lw2=True), front_a(1)]

        lwp = ppool.tile([128, BC], f32, tag="work", name="lwp")
        nc.tensor.matmul(lwp[:N_LEAVES, :], mneg, sp, start=True, stop=True)
        nc.scalar.activation(wt, lwp[:N_LEAVES, :], act.Exp)
        nc.sync.dma_start(out=wt_dram[:], in_=wt)

        # more prefilled leaves cover the exp -> wt -> wrep round trip
        prefill += [front_a(2), front_a(3), front_a(4)]
        emit_wrep_dma(0)
        emit_wrep_dma(1)
        lwg[0][1] = emit_lw2_dma(0)
        pending = [front_b(st) for st in prefill]

        # leaf_b2 contribution: out^T += b2l^T @ w^T (starts the accumulation)
        for o in range(OC):
            nc.tensor.matmul(
                pouts[o], b2l[:, o * 128 : (o + 1) * 128], wt, start=True, stop=False
            )

        # steady state: 4-leaf software-pipeline skew
        for leaf in range(5, N_LEAVES):
            pending.append(front_b(front_a(leaf)))
            leaf_out(pending.pop(0))

        # Final 4 leaves drain BANK-major: each output bank finishes all its
        # remaining accumulations consecutively, then its PSUM->SBUF copy
        # starts while later banks are still accumulating. Two batched DMAs
        # ship the halves.
        half = OC // 2
        osb = apool.tile([128, OC, BC], f32, tag="osb", bufs=1, name="osb")
        for o in range(OC):
            for idx, (p_hls, p_leaf) in enumerate(pending):
                nc.tensor.matmul(
                    pouts[o],
                    lwg[p_leaf // GL][1][:, p_leaf % GL, o * 128 : (o + 1) * 128],
                    p_hls,
                    start=False,
                    stop=(idx == len(pending) - 1),
                )
            nc.vector.tensor_copy(osb[:, o, :], pouts[o])
            if o == half - 1:
                nc.sync.dma_start(
                    out=out_d[: half * 128, :].rearrange("(o p) t -> p o t", p=128),
                    in_=osb[:, :half, :],
                )
        nc.sync.dma_start(
            out=out_d[half * 128 :, :].rearrange("(o p) t -> p o t", p=128),
            in_=osb[:, half:, :],
        )

    nc.compile()
    return nc


def _to_bf16(a: np.ndarray) -> np.ndarray:
    return np.ascontiguousarray(a, dtype=np.float32).astype(BF16)


def prep_inputs(x, node_w1, node_b1, node_w2, node_b2,
                leaf_w1, leaf_b1, leaf_w2, leaf_b2):
    """Host-side layout prep. Returns (shared weight map, per-core x maps)."""
    x = np.asarray(x, np.float32)
    node_w1 = np.asarray(node_w1, np.float32)
    node_b1 = np.asarray(node_b1, np.float32)
    node_w2 = np.asarray(node_w2, np.float32)
    node_b2 = np.asarray(node_b2, np.float32)
    leaf_w1 = np.asarray(leaf_w1, np.float32)
    leaf_b1 = np.asarray(leaf_b1, np.float32)
    leaf_w2 = np.asarray(leaf_w2, np.float32)
    leaf_b2 = np.asarray(leaf_b2, np.float32)

    # node W1 -> [128, NJ, KC*128]: (p, j, c*128+h') = W1n[c*128+p, j*128+h']
    # (W1n [768, 1008] zero-padded to 1024 columns)
    w1n_flat = node_w1.transpose(1, 0, 2).reshape(IN_DIM, HN)
    w1n_pad = np.zeros((IN_DIM, NJ * 128), np.float32)
    w1n_pad[:, :HN] = w1n_flat
    w1n = w1n_pad.reshape(KC, 128, NJ, 128).transpose(1, 2, 0, 3)
    w1n = w1n.reshape(128, NJ, KC * 128)
    # block-diagonal node W2 [HN, 63], padded to 1024 rows -> [128, NJ, 63]
    w2bd = np.zeros((NJ * 128, N_NODES), np.float32)
    for n in range(N_NODES):
        w2bd[n * NODE_HIDDEN : (n + 1) * NODE_HIDDEN, n] = node_w2[n, :, 0]
    w2bd = w2bd.reshape(NJ, 128, N_NODES).transpose(1, 0, 2)
    # node b1 -> [128, NJ]
    b1n = np.zeros((NJ * 128,), np.float32)
    b1n[:HN] = node_b1.reshape(-1)
    b1n = b1n.reshape(NJ, 128).T
    b2 = node_b2[:, 0]
    b2sp = np.stack([-b2, b2], axis=1)  # [63, 2]

    # leaf W1 grouped GL leaves per DMA: [NG, 128, GL*KC*128] with
    # (g, p, (i, c, h)) = leaf_w1[g*GL+i, c*128+p, h]
    ng = N_LEAVES // GL
    lw1 = leaf_w1.reshape(ng, GL, KC, 128, LEAF_HIDDEN).transpose(0, 3, 1, 2, 4)
    lw1 = lw1.reshape(ng, 128, GL * KC * 128)
    # leaf W2 grouped: [NG, 128, GL*OUT] with (g, p, (i, o)) = leaf_w2[g*GL+i, p, o]
    lw2 = leaf_w2.reshape(ng, GL, LEAF_HIDDEN, OUT_DIM).transpose(0, 2, 1, 3)
    lw2 = lw2.reshape(ng, 128, GL * OUT_DIM)
    b1l = leaf_b1.T  # [128, 64]

    shared = {
        "w1n": _to_bf16(w1n),
        "w2bd": _to_bf16(w2bd),
        "b1n": np.ascontiguousarray(b1n, np.float32),
        "b2sp": np.ascontiguousarray(b2sp, np.float32),
        "mneg": _path_matrix(),
        "lw1": _to_bf16(lw1),
        "b1l": np.ascontiguousarray(b1l, np.float32),
        "lw2": _to_bf16(lw2),
        "b2l": _to_bf16(leaf_b2),
    }
    xts = []
    for c in range(N_CORES):
        xc = x[c * BC : (c + 1) * BC].T  # [768, 512]
        xt = xc.reshape(KC, 128, BC).transpose(1, 0, 2)
        xts.append(_to_bf16(xt))
    return shared, xts


def kernel(**inputs) -> np.ndarray:
    global LAST_RESULT
    shared, xts = prep_inputs(**inputs)
    nc = _build_nc()
    in_maps = [{**shared, "xt": xts[c]} for c in range(N_CORES)]
    trace = os.environ.get("FFF_TRACE", "0") == "1"
    res = run_bass_kernel_spmd(nc, in_maps, list(range(N_CORES)), trace=trace)
    LAST_RESULT = res
    out = np.empty((BATCH, OUT_DIM), np.float32)
    for c in range(N_CORES):
        out[c * BC : (c + 1) * BC, :] = res.results[c]["outT"].T
    return out


# revision 58
# speedup vs baseline: 1.0038x; 1.0038x over previous
"""FFF (fast feedforward / soft MoE tree) layer for Trainium2, 8 NeuronCores.

Strategy: data-parallel over the 4096-token batch (512 tokens/core), all
weights replicated. Per core, activations live feature-major in SBUF
([feature partitions, token free-dim]) so every matmul uses native weight
slices as lhsT and 512-token tiles as rhs:

  node phase:  hn^T = relu(W1n^T x^T + b1)          8 x 6 matmuls, N=512
               z    = W2bd^T hn^T                   8 matmuls (block-diag W2)
               c    = sigmoid(z + b2)
               w^T  = exp(Mpath^T ln([c; 1-c] + eps))  ACT chain + one fp32
                                                       path-matrix matmul
  leaf phase:  per leaf l: hl = relu(W1_l^T x^T + b1_l)   6 matmuls -> PSUM
               hls = hl * w_l (per-token scale via broadcast DMA of w rows)
               out^T += W2_l^T @ hls                 6 accumulating matmuls
               (+ leaf_b2 folded in as a rank-64 matmul over w^T)

out^T [768, 512] accumulates in 6 PSUM banks across all 64 leaves (4-leaf
software-pipeline skew keeps the PE saturated; the final leaves drain
bank-major so PSUM->SBUF copies overlap the last matmuls), then two batched
DMAs write DRAM; the host transposes and concatenates the 8 core shards.
Matmul inputs are bf16 (fp32 accumulation in PSUM); the path-matrix matmul
and all bias handling stay fp32.
"""

import functools
import os
import sys
from contextlib import ExitStack

import numpy as np
import ml_dtypes

for _p in ("/opt/trn_rl_repo", "/root/.axon_site/_ro/trn_rl_repo"):
    if os.path.isdir(_p) and _p not in sys.path:
        sys.path.insert(0, _p)

import concourse.bass as bass
import concourse.tile as tile
from concourse import bacc, mybir
from concourse.bass_utils import run_bass_kernel_spmd

BF16 = ml_dtypes.bfloat16

DEPTH = 6
IN_DIM = 768
NODE_HIDDEN = 16
LEAF_HIDDEN = 128
OUT_DIM = 768
BATCH = 4096
N_NODES = 63
N_LEAVES = 64
N_CORES = 8
BC = BATCH // N_CORES          # 512 tokens per core
KC = IN_DIM // 128             # 6 contraction chunks
HN = N_NODES * NODE_HIDDEN     # 1008 node-hidden total
NJ = (HN + 127) // 128         # 8 node-hidden partition tiles (last = 112)
OC = OUT_DIM // 128            # 6 output-feature chunks
GL = 8                         # leaves per weight-DMA group (fewer DMA issues)
WG = 4                         # leaves per w-broadcast group

# Exposed for test harnesses.
LAST_RESULT = None


def _path_matrix() -> np.ndarray:
    """Mpath [128, 64]: logw = Mpath^T @ [log(c) ; pad ; log(1-c) ; pad].

    c = sigmoid(z). Row n (0..62) selects log(c_n) for leaves in the LEFT
    subtree of node n; row 64+n selects log(1-c_n) for leaves in its RIGHT
    subtree (offset 64, not 63: engine APs must start on a partition
    quadrant). Rows 63 and 127 are zero.
    """
    m = np.zeros((128, N_LEAVES), np.float32)
    for leaf in range(N_LEAVES):
        for lvl in range(DEPTH):
            node = (1 << lvl) - 1 + (leaf >> (DEPTH - lvl))
            right = (leaf >> (DEPTH - 1 - lvl)) & 1
            m[node + (64 if right else 0), leaf] = 1.0
    return m


@functools.lru_cache(maxsize=1)
def _build_nc() -> bass.Bass:
    nc = bacc.Bacc()
    f32 = mybir.dt.float32
    bf16 = mybir.dt.bfloat16

    xt_d = nc.dram_tensor("xt", [128, KC, BC], bf16, kind="ExternalInput")
    w1n_d = nc.dram_tensor("w1n", [128, NJ, KC * 128], bf16, kind="ExternalInput")
    w2bd_d = nc.dram_tensor("w2bd", [128, NJ, N_NODES], bf16, kind="ExternalInput")
    b1n_d = nc.dram_tensor("b1n", [128, NJ], f32, kind="ExternalInput")
    b2sp_d = nc.dram_tensor("b2sp", [N_NODES, 2], f32, kind="ExternalInput")
    mneg_d = nc.dram_tensor("mneg", [128, N_LEAVES], f32, kind="ExternalInput")
    lw1_d = nc.dram_tensor(
        "lw1", [N_LEAVES // GL, 128, GL * KC * 128], bf16, kind="ExternalInput"
    )
    b1l_d = nc.dram_tensor("b1l", [128, N_LEAVES], f32, kind="ExternalInput")
    lw2_d = nc.dram_tensor(
        "lw2", [N_LEAVES // GL, 128, GL * OUT_DIM], bf16, kind="ExternalInput"
    )
    b2l_d = nc.dram_tensor("b2l", [N_LEAVES, OUT_DIM], bf16, kind="ExternalInput")
    out_d = nc.dram_tensor("outT", [OUT_DIM, BC], f32, kind="ExternalOutput")
    # Staging buffer so the per-token leaf weights can be broadcast-read
    # (partition-step-0 APs need a DRAM source).
    wt_dram = nc.dram_tensor("wt_scratch", [N_LEAVES, BC], bf16)

    act = mybir.ActivationFunctionType

    with tile.TileContext(nc) as tc, ExitStack() as ctx:
        consts = ctx.enter_context(tc.tile_pool(name="consts", bufs=1))
        wpool = ctx.enter_context(tc.tile_pool(name="wpool", bufs=3))
        apool = ctx.enter_context(tc.tile_pool(name="apool", bufs=2))
        ppool = ctx.enter_context(tc.tile_pool(name="ppool", bufs=2, space="PSUM"))
        opool = ctx.enter_context(tc.tile_pool(name="opool", bufs=1, space="PSUM"))

        # Every dma_start costs ~0.6us of serial issue time on the issuing
        # sequencer, so: x in ONE dma, node weights in 3 (j0 / j1 / j2-7 --
        # sized so each chunk lands just before PE needs it), everything not
        # needed immediately issued from the otherwise-idle GpSimd sequencer.
        xt = consts.tile([128, KC, BC], bf16)
        w1n = consts.tile([128, NJ, KC, 128], bf16)
        nc.sync.dma_start(out=xt[:, 0:1, :], in_=xt_d[:, 0:1, :])
        nc.sync.dma_start(
            out=w1n[:, 0, :, :],
            in_=w1n_d[:, 0, :].rearrange("p (c h) -> p c h", c=KC),
        )
        nc.sync.dma_start(out=xt[:, 1:3, :], in_=xt_d[:, 1:3, :])
        nc.sync.dma_start(out=xt[:, 3:, :], in_=xt_d[:, 3:, :])
        nc.sync.dma_start(
            out=w1n[:, 1, :, :],
            in_=w1n_d[:, 1, :].rearrange("p (c h) -> p c h", c=KC),
        )
        nc.sync.dma_start(
            out=w1n[:, 2:NJ, :, :],
            in_=w1n_d[:, 2:NJ, :].rearrange("p j (c h) -> p j c h", c=KC),
        )
        w2bd = consts.tile([128, NJ, N_NODES], bf16)
        nc.gpsimd.dma_start(out=w2bd, in_=w2bd_d[:])
        b1n = consts.tile([128, NJ], f32)
        nc.gpsimd.dma_start(out=b1n, in_=b1n_d[:])
        b2sp = consts.tile([N_NODES, 2], f32)
        nc.gpsimd.dma_start(out=b2sp, in_=b2sp_d[:])
        mneg = consts.tile([128, N_LEAVES], f32)
        nc.gpsimd.dma_start(out=mneg, in_=mneg_d[:])
        b1l = consts.tile([128, N_LEAVES], f32)
        nc.gpsimd.dma_start(out=b1l, in_=b1l_d[:])
        b2l = consts.tile([N_LEAVES, OUT_DIM], bf16)
        nc.gpsimd.dma_start(out=b2l, in_=b2l_d[:])

        hn = consts.tile([128, NJ, BC], bf16)
        pre = consts.tile([128, BC], f32)
        sp = consts.tile([128, BC], f32)
        wt = consts.tile([N_LEAVES, BC], bf16)
        # rows 63/127 of pre stay 1.0 -> ln gives 0 there, and Mpath's zero
        # rows ignore them
        nc.vector.memset(pre, 1.0)
        epsb = consts.tile([128, 1], f32)
        nc.vector.memset(epsb, 1e-38)

        # PE warmup: the HAM clock gate keeps an idle PE at 1.2 GHz and only
        # releases to 2.4 GHz after ~3.4us of sustained activity. The PE sits
        # idle waiting for the first DMAs anyway, so burn that window with
        # dummy 1x1 matmuls to arrive at the first real matmul already warm.
        warm = ppool.tile([128, BC], f32, tag="work", name="warm")
        for _ in range(8):
            nc.tensor.matmul(warm[:1, :], epsb, pre, start=True, stop=True)

        # ---- node phase: gate pre-activations z, then leaf weights w ----
        for j in range(NJ):
            pj = min(128, HN - j * 128)
            ph = ppool.tile([128, BC], f32, tag="work")
            for c in range(KC):
                nc.tensor.matmul(
                    ph[:pj, :],
                    w1n[:, j, c, :pj],
                    xt[:, c, :],
                    start=(c == 0),
                    stop=(c == KC - 1),
                )
            nc.scalar.activation(
                hn[:pj, j, :], ph[:pj, :], act.Relu, bias=b1n[:pj, j : j + 1]
            )

        zp = ppool.tile([128, BC], f32, tag="work")
        for j in range(NJ):
            pj = min(128, HN - j * 128)
            nc.tensor.matmul(
                zp[:N_NODES, :],
                w2bd[:pj, j, :],
                hn[:pj, j, :],
                start=(j == 0),
                stop=(j == NJ - 1),
            )
        # ---- leaf-phase pipeline helpers ----
        pouts = [
            opool.tile([128, BC], f32, tag=f"out{o}", name=f"pout{o}")
            for o in range(OC)
        ]
        wreps = {}

        def emit_wrep_dma(grp):
            """Broadcast leaf-weight rows (4 leaves) across all partitions."""
            wrep = wpool.tile([128, WG, BC], bf16, tag="wrep", bufs=3, name="wrep")
            src = bass.AP(
                tensor=wt_dram,
                offset=grp * WG * BC,
                ap=[[0, 128], [BC, WG], [1, BC]],
            )
            nc.sync.dma_start(out=wrep, in_=src)
            wreps[grp] = wrep

        lwg = {}

        def emit_lw1_dma(g):
            w1t = wpool.tile([128, GL, KC, 128], bf16, tag="lw1", bufs=2, name="w1t")
            nc.sync.dma_start(
                out=w1t,
                in_=lw1_d[g].rearrange("p (i c h) -> p i c h", i=GL, c=KC),
            )
            return w1t

        def emit_lw2_dma(g):
            w2t = wpool.tile([128, GL, OUT_DIM], bf16, tag="lw2", bufs=2, name="w2t")
            nc.sync.dma_start(
                out=w2t, in_=lw2_d[g].rearrange("p (i o) -> p i o", i=GL)
            )
            return w2t

        def front_a(leaf, defer_lw2=False):
            """Weight DMAs (grouped) + hl matmuls + relu for one leaf."""
            if leaf % WG == 2 and leaf >= WG and leaf + 2 < N_LEAVES:
                # prefetch the NEXT group's broadcast two leaves early
                # (groups 0/1 are emitted explicitly after wt_dram is
                # written; emission order carries the RAW dep on wt_dram)
                emit_wrep_dma(leaf // WG + 1)
            g = leaf // GL
            if leaf % GL == 0:
                lwg[g] = [emit_lw1_dma(g), None if defer_lw2 else emit_lw2_dma(g)]
            w1t = lwg[g][0]
            i = leaf % GL

            ph = ppool.tile([128, BC], f32, tag="work", name="ph")
            for c in range(KC):
                nc.tensor.matmul(
                    ph,
                    w1t[:, i, c, :],
                    xt[:, c, :],
                    start=(c == 0),
                    stop=(c == KC - 1),
                )
            hl = apool.tile([128, BC], bf16, tag="hl", bufs=6, name="hl")
            nc.scalar.activation(hl, ph, act.Relu, bias=b1l[:, leaf : leaf + 1])
            return (hl, leaf)

        def front_b(st):
            """Per-token leaf-weight scale (needs wrep for the leaf's group)."""
            hl, leaf = st
            hls = apool.tile([128, BC], bf16, tag="hls", bufs=7, name="hls")
            nc.vector.tensor_mul(hls, hl, wreps[leaf // WG][:, leaf % WG, :])
            return (hls, leaf)

        def leaf_out(pend, last=False):
            p_hls, p_leaf = pend
            p_w2t = lwg[p_leaf // GL][1]
            for o in range(OC):
                nc.tensor.matmul(
                    pouts[o],
                    p_w2t[:, p_leaf % GL, o * 128 : (o + 1) * 128],
                    p_hls,
                    start=False,
                    stop=last,
                )

        # The gating ACT/DVE chain is emitted BEFORE the prefilled leaves so
        # it isn't queued behind their relu ops on the Scalar engine.
        # pre[0:63]  = c = sigmoid(zp + b2);  pre[64:127] = 1 - c
        # sp = ln(pre + 1e-38) in ONE activation over all 128 partitions --
        # the +eps bias keeps saturated gates finite (ln(1e-38) = -87.5,
        # whose exp underflows to the correct 0 leaf weight, and never
        # produces inf/NaN in the path matmul).
        nc.scalar.activation(
            pre[0:N_NODES, :], zp[:N_NODES, :], act.Sigmoid, bias=b2sp[:, 1:2]
        )
        nc.vector.tensor_scalar(
            pre[64 : 64 + N_NODES, :], pre[0:N_NODES, :], -1.0, 1.0,
            mybir.AluOpType.mult, mybir.AluOpType.add,
        )
        nc.scalar.activation(sp, pre, act.Ln, bias=epsb)

        # Prefill leaves: their hl matmuls keep PE busy while the gating
        # chain (sigmoid/ln table loads -> path matmul -> exp -> DRAM round
        # trip for the broadcast) produces the leaf weights. Group 0's
        # second-matmul weights are deferred so the wt round trip doesn't
        # queue behind their transfer.
        prefill = [front_a(0, defer_lw2=True), front_a(1)]

        lwp = ppool.tile([128, BC], f32, tag="work", name="lwp")
        nc.tensor.matmul(lwp[:N_LEAVES, :], mneg, sp, start=True, stop=True)
        nc.scalar.activation(wt, lwp[:N_LEAVES, :], act.Exp)
        nc.sync.dma_start(out=wt_dram[:], in_=wt)

        # more prefilled leaves cover the exp -> wt -> wrep round trip
        prefill += [front_a(2), front_a(3), front_a(4)]
        emit_wrep_dma(0)
        emit_wrep_dma(1)
        lwg[0][1] = emit_lw2_dma(0)
        pending = [front_b(st) for st in prefill]

        # leaf_b2 contribution: out^T += b2l^T @ w^T (starts the accumulation)
        for o in range(OC):
            nc.tensor.matmul(
                pouts[o], b2l[:, o * 128 : (o + 1) * 128], wt, start=True, stop=False
            )

        # steady state: 4-leaf software-pipeline skew
        for leaf in range(5, N_LEAVES):
            pending.append(front_b(front_a(leaf)))
            leaf_out(pending.pop(0))

        # Final 4 leaves drain BANK-major: each output bank finishes all its
        # remaining accumulations consecutively, then its PSUM->SBUF copy
        # starts while later banks are still accumulating. Two batched DMAs
        # ship the halves.
        half = OC // 2
        osb = apool.tile([128, OC, BC], f32, tag="osb", bufs=1, name="osb")
        for o in range(OC):
            for idx, (p_hls, p_leaf) in enumerate(pending):
                nc.tensor.matmul(
                    pouts[o],
                    lwg[p_leaf // GL][1][:, p_leaf % GL, o * 128 : (o + 1) * 128],
                    p_hls,
                    start=False,
                    stop=(idx == len(pending) - 1),
                )
            nc.vector.tensor_copy(osb[:, o, :], pouts[o])
            if o == half - 1:
                nc.sync.dma_start(
                    out=out_d[: half * 128, :].rearrange("(o p) t -> p o t", p=128),
                    in_=osb[:, :half, :],
                )
        nc.sync.dma_start(
            out=out_d[half * 128 :, :].rearrange("(o p) t -> p o t", p=128),
            in_=osb[:, half:, :],
        )

    nc.compile()
    return nc


def _to_bf16(a: np.ndarray) -> np.ndarray:
    return np.ascontiguousarray(a, dtype=np.float32).astype(BF16)


def prep_inputs(x, node_w1, node_b1, node_w2, node_b2,
                leaf_w1, leaf_b1, leaf_w2, leaf_b2):
    """Host-side layout prep. Returns (shared weight map, per-core x maps)."""
    x = np.asarray(x, np.float32)
    node_w1 = np.asarray(node_w1, np.float32)
    node_b1 = np.asarray(node_b1, np.float32)
    node_w2 = np.asarray(node_w2, np.float32)
    node_b2 = np.asarray(node_b2, np.float32)
    leaf_w1 = np.asarray(leaf_w1, np.float32)
    leaf_b1 = np.asarray(leaf_b1, np.float32)
    leaf_w2 = np.asarray(leaf_w2, np.float32)
    leaf_b2 = np.asarray(leaf_b2, np.float32)

    # node W1 -> [128, NJ, KC*128]: (p, j, c*128+h') = W1n[c*128+p, j*128+h']
    # (W1n [768, 1008] zero-padded to 1024 columns)
    w1n_flat = node_w1.transpose(1, 0, 2).reshape(IN_DIM, HN)
    w1n_pad = np.zeros((IN_DIM, NJ * 128), np.float32)
    w1n_pad[:, :HN] = w1n_flat
    w1n = w1n_pad.reshape(KC, 128, NJ, 128).transpose(1, 2, 0, 3)
    w1n = w1n.reshape(128, NJ, KC * 128)
    # block-diagonal node W2 [HN, 63], padded to 1024 rows -> [128, NJ, 63]
    w2bd = np.zeros((NJ * 128, N_NODES), np.float32)
    for n in range(N_NODES):
        w2bd[n * NODE_HIDDEN : (n + 1) * NODE_HIDDEN, n] = node_w2[n, :, 0]
    w2bd = w2bd.reshape(NJ, 128, N_NODES).transpose(1, 0, 2)
    # node b1 -> [128, NJ]
    b1n = np.zeros((NJ * 128,), np.float32)
    b1n[:HN] = node_b1.reshape(-1)
    b1n = b1n.reshape(NJ, 128).T
    b2 = node_b2[:, 0]
    b2sp = np.stack([-b2, b2], axis=1)  # [63, 2]

    # leaf W1 grouped GL leaves per DMA: [NG, 128, GL*KC*128] with
    # (g, p, (i, c, h)) = leaf_w1[g*GL+i, c*128+p, h]
    ng = N_LEAVES // GL
    lw1 = leaf_w1.reshape(ng, GL, KC, 128, LEAF_HIDDEN).transpose(0, 3, 1, 2, 4)
    lw1 = lw1.reshape(ng, 128, GL * KC * 128)
    # leaf W2 grouped: [NG, 128, GL*OUT] with (g, p, (i, o)) = leaf_w2[g*GL+i, p, o]
    lw2 = leaf_w2.reshape(ng, GL, LEAF_HIDDEN, OUT_DIM).transpose(0, 2, 1, 3)
    lw2 = lw2.reshape(ng, 128, GL * OUT_DIM)
    b1l = leaf_b1.T  # [128, 64]

    shared = {
        "w1n": _to_bf16(w1n),
        "w2bd": _to_bf16(w2bd),
        "b1n": np.ascontiguousarray(b1n, np.float32),
        "b2sp": np.ascontiguousarray(b2sp, np.float32),
        "mneg": _path_matrix(),
        "lw1": _to_bf16(lw1),
        "b1l": np.ascontiguousarray(b1l, np.float32),
        "lw2": _to_bf16(lw2),
        "b2l": _to_bf16(leaf_b2),
    }
    xts = []
    for c in range(N_CORES):
        xc = x[c * BC : (c + 1) * BC].T  # [768, 512]
        xt = xc.reshape(KC, 128, BC).transpose(1, 0, 2)
        xts.append(_to_bf16(xt))
    return shared, xts


def kernel(**inputs) -> np.ndarray:
    global LAST_RESULT
    shared, xts = prep_inputs(**inputs)
    nc = _build_nc()
    in_maps = [{**shared, "xt": xts[c]} for c in range(N_CORES)]
    trace = os.environ.get("FFF_TRACE", "0") == "1"
    res = run_bass_kernel_spmd(nc, in_maps, list(range(N_CORES)), trace=trace)
    LAST_RESULT = res
    out = np.empty((BATCH, OUT_DIM), np.float32)
    for c in range(N_CORES):
        out[c * BC : (c + 1) * BC, :] = res.results[c]["outT"].T
    return out


# revision 59
# speedup vs baseline: 1.0054x; 1.0016x over previous
"""FFF (fast feedforward / soft MoE tree) layer for Trainium2, 8 NeuronCores.

Strategy: data-parallel over the 4096-token batch (512 tokens/core), all
weights replicated. Per core, activations live feature-major in SBUF
([feature partitions, token free-dim]) so every matmul uses native weight
slices as lhsT and 512-token tiles as rhs:

  node phase:  hn^T = relu(W1n^T x^T + b1)          8 x 6 matmuls, N=512
               z    = W2bd^T hn^T                   8 matmuls (block-diag W2)
               c    = sigmoid(z + b2)
               w^T  = exp(Mpath^T ln([c; 1-c] + eps))  ACT chain + one fp32
                                                       path-matrix matmul
  leaf phase:  per leaf l: hl = relu(W1_l^T x^T + b1_l)   6 matmuls -> PSUM
               hls = hl * w_l (per-token scale via broadcast DMA of w rows)
               out^T += W2_l^T @ hls                 6 accumulating matmuls
               (+ leaf_b2 folded in as a rank-64 matmul over w^T)

out^T [768, 512] accumulates in 6 PSUM banks across all 64 leaves (4-leaf
software-pipeline skew keeps the PE saturated; the final leaves drain
bank-major so PSUM->SBUF copies overlap the last matmuls), then two batched
DMAs write DRAM; the host transposes and concatenates the 8 core shards.
Matmul inputs are bf16 (fp32 accumulation in PSUM); the path-matrix matmul
and all bias handling stay fp32.
"""

import functools
import os
import sys
from contextlib import ExitStack

import numpy as np
import ml_dtypes

for _p in ("/opt/trn_rl_repo", "/root/.axon_site/_ro/trn_rl_repo"):
    if os.path.isdir(_p) and _p not in sys.path:
        sys.path.insert(0, _p)

import concourse.bass as bass
import concourse.tile as tile
from concourse import bacc, mybir
from concourse.bass_utils import run_bass_kernel_spmd

BF16 = ml_dtypes.bfloat16

DEPTH = 6
IN_DIM = 768
NODE_HIDDEN = 16
LEAF_HIDDEN = 128
OUT_DIM = 768
BATCH = 4096
N_NODES = 63
N_LEAVES = 64
N_CORES = 8
BC = BATCH // N_CORES          # 512 tokens per core
KC = IN_DIM // 128             # 6 contraction chunks
HN = N_NODES * NODE_HIDDEN     # 1008 node-hidden total
NJ = (HN + 127) // 128         # 8 node-hidden partition tiles (last = 112)
OC = OUT_DIM // 128            # 6 output-feature chunks
GL = 8                         # leaves per weight-DMA group (fewer DMA issues)
WG = 4                         # leaves per w-broadcast group

# Exposed for test harnesses.
LAST_RESULT = None


def _path_matrix() -> np.ndarray:
    """Mpath [128, 64]: logw = Mpath^T @ [log(c) ; pad ; log(1-c) ; pad].

    c = sigmoid(z). Row n (0..62) selects log(c_n) for leaves in the LEFT
    subtree of node n; row 64+n selects log(1-c_n) for leaves in its RIGHT
    subtree (offset 64, not 63: engine APs must start on a partition
    quadrant). Rows 63 and 127 are zero.
    """
    m = np.zeros((128, N_LEAVES), np.float32)
    for leaf in range(N_LEAVES):
        for lvl in range(DEPTH):
            node = (1 << lvl) - 1 + (leaf >> (DEPTH - lvl))
            right = (leaf >> (DEPTH - 1 - lvl)) & 1
            m[node + (64 if right else 0), leaf] = 1.0
    return m


@functools.lru_cache(maxsize=1)
def _build_nc() -> bass.Bass:
    nc = bacc.Bacc()
    f32 = mybir.dt.float32
    bf16 = mybir.dt.bfloat16

    xt_d = nc.dram_tensor("xt", [128, KC, BC], bf16, kind="ExternalInput")
    w1n_d = nc.dram_tensor("w1n", [128, NJ, KC * 128], bf16, kind="ExternalInput")
    w2bd_d = nc.dram_tensor("w2bd", [128, NJ, N_NODES], bf16, kind="ExternalInput")
    b1n_d = nc.dram_tensor("b1n", [128, NJ], f32, kind="ExternalInput")
    b2sp_d = nc.dram_tensor("b2sp", [N_NODES, 2], f32, kind="ExternalInput")
    mneg_d = nc.dram_tensor("mneg", [128, N_LEAVES], f32, kind="ExternalInput")
    lw1_d = nc.dram_tensor(
        "lw1", [N_LEAVES // GL, 128, GL * KC * 128], bf16, kind="ExternalInput"
    )
    b1l_d = nc.dram_tensor("b1l", [128, N_LEAVES], f32, kind="ExternalInput")
    lw2_d = nc.dram_tensor(
        "lw2", [N_LEAVES // GL, 128, GL * OUT_DIM], bf16, kind="ExternalInput"
    )
    b2l_d = nc.dram_tensor("b2l", [N_LEAVES, OUT_DIM], bf16, kind="ExternalInput")
    out_d = nc.dram_tensor("outT", [OUT_DIM, BC], f32, kind="ExternalOutput")
    # Staging buffer so the per-token leaf weights can be broadcast-read
    # (partition-step-0 APs need a DRAM source).
    wt_dram = nc.dram_tensor("wt_scratch", [N_LEAVES, BC], bf16)

    act = mybir.ActivationFunctionType

    with tile.TileContext(nc) as tc, ExitStack() as ctx:
        consts = ctx.enter_context(tc.tile_pool(name="consts", bufs=1))
        wpool = ctx.enter_context(tc.tile_pool(name="wpool", bufs=3))
        apool = ctx.enter_context(tc.tile_pool(name="apool", bufs=2))
        ppool = ctx.enter_context(tc.tile_pool(name="ppool", bufs=2, space="PSUM"))
        opool = ctx.enter_context(tc.tile_pool(name="opool", bufs=1, space="PSUM"))

        # Every dma_start costs ~0.6us of serial issue time on the issuing
        # sequencer, so: x in ONE dma, node weights in 3 (j0 / j1 / j2-7 --
        # sized so each chunk lands just before PE needs it), everything not
        # needed immediately issued from the otherwise-idle GpSimd sequencer.
        xt = consts.tile([128, KC, BC], bf16)
        w1n = consts.tile([128, NJ, KC, 128], bf16)
        nc.sync.dma_start(out=xt[:, 0:1, :], in_=xt_d[:, 0:1, :])
        nc.sync.dma_start(
            out=w1n[:, 0, :, :],
            in_=w1n_d[:, 0, :].rearrange("p (c h) -> p c h", c=KC),
        )
        nc.sync.dma_start(out=xt[:, 1:3, :], in_=xt_d[:, 1:3, :])
        nc.sync.dma_start(out=xt[:, 3:, :], in_=xt_d[:, 3:, :])
        nc.sync.dma_start(
            out=w1n[:, 1, :, :],
            in_=w1n_d[:, 1, :].rearrange("p (c h) -> p c h", c=KC),
        )
        nc.sync.dma_start(
            out=w1n[:, 2:NJ, :, :],
            in_=w1n_d[:, 2:NJ, :].rearrange("p j (c h) -> p j c h", c=KC),
        )
        w2bd = consts.tile([128, NJ, N_NODES], bf16)
        nc.gpsimd.dma_start(out=w2bd, in_=w2bd_d[:])
        b1n = consts.tile([128, NJ], f32)
        nc.gpsimd.dma_start(out=b1n, in_=b1n_d[:])
        b2sp = consts.tile([N_NODES, 2], f32)
        nc.gpsimd.dma_start(out=b2sp, in_=b2sp_d[:])
        mneg = consts.tile([128, N_LEAVES], f32)
        nc.gpsimd.dma_start(out=mneg, in_=mneg_d[:])
        b1l = consts.tile([128, N_LEAVES], f32)
        nc.gpsimd.dma_start(out=b1l, in_=b1l_d[:])
        b2l = consts.tile([N_LEAVES, OUT_DIM], bf16)
        nc.gpsimd.dma_start(out=b2l, in_=b2l_d[:])

        hn = consts.tile([128, NJ, BC], bf16)
        pre = consts.tile([128, BC], f32)
        sp = consts.tile([128, BC], f32)
        wt = consts.tile([N_LEAVES, BC], bf16)
        # rows 63/127 of pre stay 1.0 -> ln gives 0 there, and Mpath's zero
        # rows ignore them
        nc.vector.memset(pre, 1.0)
        epsb = consts.tile([128, 1], f32)
        nc.vector.memset(epsb, 1e-38)

        # PE warmup: the HAM clock gate keeps an idle PE at 1.2 GHz and only
        # releases to 2.4 GHz after ~3.4us of sustained activity. The PE sits
        # idle waiting for the first DMAs anyway, so burn that window with
        # dummy 1x1 matmuls to arrive at the first real matmul already warm.
        warm = ppool.tile([128, BC], f32, tag="work", name="warm")
        for _ in range(8):
            nc.tensor.matmul(warm[:1, :], epsb, pre, start=True, stop=True)

        # ---- node phase: gate pre-activations z, then leaf weights w ----
        for j in range(NJ):
            pj = min(128, HN - j * 128)
            ph = ppool.tile([128, BC], f32, tag="work")
            for c in range(KC):
                nc.tensor.matmul(
                    ph[:pj, :],
                    w1n[:, j, c, :pj],
                    xt[:, c, :],
                    start=(c == 0),
                    stop=(c == KC - 1),
                )
            nc.scalar.activation(
                hn[:pj, j, :], ph[:pj, :], act.Relu, bias=b1n[:pj, j : j + 1]
            )

        zp = ppool.tile([128, BC], f32, tag="work")
        for j in range(NJ):
            pj = min(128, HN - j * 128)
            nc.tensor.matmul(
                zp[:N_NODES, :],
                w2bd[:pj, j, :],
                hn[:pj, j, :],
                start=(j == 0),
                stop=(j == NJ - 1),
            )
        # ---- leaf-phase pipeline helpers ----
        pouts = [
            opool.tile([128, BC], f32, tag=f"out{o}", name=f"pout{o}")
            for o in range(OC)
        ]
        wreps = {}

        def emit_wrep_dma(grp):
            """Broadcast leaf-weight rows (4 leaves) across all partitions."""
            wrep = wpool.tile([128, WG, BC], bf16, tag="wrep", bufs=3, name="wrep")
            src = bass.AP(
                tensor=wt_dram,
                offset=grp * WG * BC,
                ap=[[0, 128], [BC, WG], [1, BC]],
            )
            nc.sync.dma_start(out=wrep, in_=src)
            wreps[grp] = wrep

        lwg = {}

        def emit_lw1_dma(g):
            w1t = wpool.tile([128, GL, KC, 128], bf16, tag="lw1", bufs=2, name="w1t")
            nc.sync.dma_start(
                out=w1t,
                in_=lw1_d[g].rearrange("p (i c h) -> p i c h", i=GL, c=KC),
            )
            return w1t

        def emit_lw2_dma(g):
            w2t = wpool.tile([128, GL, OUT_DIM], bf16, tag="lw2", bufs=2, name="w2t")
            nc.sync.dma_start(
                out=w2t, in_=lw2_d[g].rearrange("p (i o) -> p i o", i=GL)
            )
            return w2t

        def front_a(leaf, defer_lw2=False):
            """Weight DMAs (grouped) + hl matmuls + relu for one leaf."""
            if leaf % WG == 2 and leaf >= WG and leaf + 2 < N_LEAVES:
                # prefetch the NEXT group's broadcast two leaves early
                # (groups 0/1 are emitted explicitly after wt_dram is
                # written; emission order carries the RAW dep on wt_dram)
                emit_wrep_dma(leaf // WG + 1)
            g = leaf // GL
            if leaf % GL == 0:
                lwg[g] = [emit_lw1_dma(g), None if defer_lw2 else emit_lw2_dma(g)]
            w1t = lwg[g][0]
            i = leaf % GL

            ph = ppool.tile([128, BC], f32, tag="work", name="ph")
            for c in range(KC):
                nc.tensor.matmul(
                    ph,
                    w1t[:, i, c, :],
                    xt[:, c, :],
                    start=(c == 0),
                    stop=(c == KC - 1),
                )
            hl = apool.tile([128, BC], bf16, tag="hl", bufs=6, name="hl")
            nc.scalar.activation(hl, ph, act.Relu, bias=b1l[:, leaf : leaf + 1])
            return (hl, leaf)

        def front_b(st):
            """Per-token leaf-weight scale (needs wrep for the leaf's group)."""
            hl, leaf = st
            hls = apool.tile([128, BC], bf16, tag="hls", bufs=7, name="hls")
            nc.vector.tensor_mul(hls, hl, wreps[leaf // WG][:, leaf % WG, :])
            return (hls, leaf)

        def leaf_out(pend, last=False):
            p_hls, p_leaf = pend
            p_w2t = lwg[p_leaf // GL][1]
            for o in range(OC):
                nc.tensor.matmul(
                    pouts[o],
                    p_w2t[:, p_leaf % GL, o * 128 : (o + 1) * 128],
                    p_hls,
                    start=False,
                    stop=last,
                )

        # The gating ACT/DVE chain is emitted BEFORE the prefilled leaves so
        # it isn't queued behind their relu ops on the Scalar engine.
        # pre[0:63]  = c = sigmoid(zp + b2);  pre[64:127] = 1 - c
        # sp = ln(pre + 1e-38) in ONE activation over all 128 partitions --
        # the +eps bias keeps saturated gates finite (ln(1e-38) = -87.5,
        # whose exp underflows to the correct 0 leaf weight, and never
        # produces inf/NaN in the path matmul).
        nc.scalar.activation(
            pre[0:N_NODES, :], zp[:N_NODES, :], act.Sigmoid, bias=b2sp[:, 1:2]
        )
        nc.vector.tensor_scalar(
            pre[64 : 64 + N_NODES, :], pre[0:N_NODES, :], -1.0, 1.0,
            mybir.AluOpType.mult, mybir.AluOpType.add,
        )
        nc.scalar.activation(sp, pre, act.Ln, bias=epsb)

        # Prefill leaves: their hl matmuls keep PE busy while the gating
        # chain (sigmoid/ln table loads -> path matmul -> exp -> DRAM round
        # trip for the broadcast) produces the leaf weights. Group 0's
        # second-matmul weights are deferred so the wt round trip doesn't
        # queue behind their transfer.
        prefill = [front_a(0, defer_lw2=True), front_a(1)]

        lwp = ppool.tile([128, BC], f32, tag="work", name="lwp")
        nc.tensor.matmul(lwp[:N_LEAVES, :], mneg, sp, start=True, stop=True)
        nc.scalar.activation(wt, lwp[:N_LEAVES, :], act.Exp)
        nc.sync.dma_start(out=wt_dram[:], in_=wt)

        # more prefilled leaves cover the exp -> wt -> wrep round trip
        prefill += [front_a(2), front_a(3), front_a(4)]
        emit_wrep_dma(0)
        emit_wrep_dma(1)
        lwg[0][1] = emit_lw2_dma(0)
        pending = [front_b(st) for st in prefill]

        # leaf_b2 contribution: out^T += b2l^T @ w^T (starts the accumulation)
        for o in range(OC):
            nc.tensor.matmul(
                pouts[o], b2l[:, o * 128 : (o + 1) * 128], wt, start=True, stop=False
            )

        # steady state: 4-leaf software-pipeline skew
        for leaf in range(5, N_LEAVES):
            pending.append(front_b(front_a(leaf)))
            leaf_out(pending.pop(0))

        # Final leaves drain BANK-major: each output bank finishes all its
        # remaining accumulations consecutively, then its PSUM->SBUF copy
        # starts while later banks are still accumulating. Output DMAs ship
        # in three pieces (3/2/1 banks) so the LAST bank -- the critical
        # path after the final matmul -- rides a small 256KB transfer.
        osb = apool.tile([128, OC, BC], f32, tag="osb", bufs=1, name="osb")
        dma_after = {2: (0, 3), 4: (3, 5), 5: (5, 6)}
        for o in range(OC):
            for idx, (p_hls, p_leaf) in enumerate(pending):
                nc.tensor.matmul(
                    pouts[o],
                    lwg[p_leaf // GL][1][:, p_leaf % GL, o * 128 : (o + 1) * 128],
                    p_hls,
                    start=False,
                    stop=(idx == len(pending) - 1),
                )
            nc.vector.tensor_copy(osb[:, o, :], pouts[o])
            if o in dma_after:
                lo, hi = dma_after[o]
                nc.sync.dma_start(
                    out=out_d[lo * 128 : hi * 128, :].rearrange(
                        "(o p) t -> p o t", p=128
                    ),
                    in_=osb[:, lo:hi, :],
                )

    nc.compile()
    return nc


def _to_bf16(a: np.ndarray) -> np.ndarray:
    return np.ascontiguousarray(a, dtype=np.float32).astype(BF16)


def prep_inputs(x, node_w1, node_b1, node_w2, node_b2,
                leaf_w1, leaf_b1, leaf_w2, leaf_b2):
    """Host-side layout prep. Returns (shared weight map, per-core x maps)."""
    x = np.asarray(x, np.float32)
    node_w1 = np.asarray(node_w1, np.float32)
    node_b1 = np.asarray(node_b1, np.float32)
    node_w2 = np.asarray(node_w2, np.float32)
    node_b2 = np.asarray(node_b2, np.float32)
    leaf_w1 = np.asarray(leaf_w1, np.float32)
    leaf_b1 = np.asarray(leaf_b1, np.float32)
    leaf_w2 = np.asarray(leaf_w2, np.float32)
    leaf_b2 = np.asarray(leaf_b2, np.float32)

    # node W1 -> [128, NJ, KC*128]: (p, j, c*128+h') = W1n[c*128+p, j*128+h']
    # (W1n [768, 1008] zero-padded to 1024 columns)
    w1n_flat = node_w1.transpose(1, 0, 2).reshape(IN_DIM, HN)
    w1n_pad = np.zeros((IN_DIM, NJ * 128), np.float32)
    w1n_pad[:, :HN] = w1n_flat
    w1n = w1n_pad.reshape(KC, 128, NJ, 128).transpose(1, 2, 0, 3)
    w1n = w1n.reshape(128, NJ, KC * 128)
    # block-diagonal node W2 [HN, 63], padded to 1024 rows -> [128, NJ, 63]
    w2bd = np.zeros((NJ * 128, N_NODES), np.float32)
    for n in range(N_NODES):
        w2bd[n * NODE_HIDDEN : (n + 1) * NODE_HIDDEN, n] = node_w2[n, :, 0]
    w2bd = w2bd.reshape(NJ, 128, N_NODES).transpose(1, 0, 2)
    # node b1 -> [128, NJ]
    b1n = np.zeros((NJ * 128,), np.float32)
    b1n[:HN] = node_b1.reshape(-1)
    b1n = b1n.reshape(NJ, 128).T
    b2 = node_b2[:, 0]
    b2sp = np.stack([-b2, b2], axis=1)  # [63, 2]

    # leaf W1 grouped GL leaves per DMA: [NG, 128, GL*KC*128] with
    # (g, p, (i, c, h)) = leaf_w1[g*GL+i, c*128+p, h]
    ng = N_LEAVES // GL
    lw1 = leaf_w1.reshape(ng, GL, KC, 128, LEAF_HIDDEN).transpose(0, 3, 1, 2, 4)
    lw1 = lw1.reshape(ng, 128, GL * KC * 128)
    # leaf W2 grouped: [NG, 128, GL*OUT] with (g, p, (i, o)) = leaf_w2[g*GL+i, p, o]
    lw2 = leaf_w2.reshape(ng, GL, LEAF_HIDDEN, OUT_DIM).transpose(0, 2, 1, 3)
    lw2 = lw2.reshape(ng, 128, GL * OUT_DIM)
    b1l = leaf_b1.T  # [128, 64]

    shared = {
        "w1n": _to_bf16(w1n),
        "w2bd": _to_bf16(w2bd),
        "b1n": np.ascontiguousarray(b1n, np.float32),
        "b2sp": np.ascontiguousarray(b2sp, np.float32),
        "mneg": _path_matrix(),
        "lw1": _to_bf16(lw1),
        "b1l": np.ascontiguousarray(b1l, np.float32),
        "lw2": _to_bf16(lw2),
        "b2l": _to_bf16(leaf_b2),
    }
    xts = []
    for c in range(N_CORES):
        xc = x[c * BC : (c + 1) * BC].T  # [768, 512]
        xt = xc.reshape(KC, 128, BC).transpose(1, 0, 2)
        xts.append(_to_bf16(xt))
    return shared, xts


def kernel(**inputs) -> np.ndarray:
    global LAST_RESULT
    shared, xts = prep_inputs(**inputs)
    nc = _build_nc()
    in_maps = [{**shared, "xt": xts[c]} for c in range(N_CORES)]
    trace = os.environ.get("FFF_TRACE", "0") == "1"
    res = run_bass_kernel_spmd(nc, in_maps, list(range(N_CORES)), trace=trace)
    LAST_RESULT = res
    out = np.empty((BATCH, OUT_DIM), np.float32)
    for c in range(N_CORES):
        out[c * BC : (c + 1) * BC, :] = res.results[c]["outT"].T
    return out
